# revision 16
# baseline (speedup 1.0000x reference)
"""Trainium2 Bass kernel for a two-LSTM premise/hypothesis classifier.

Model (per reference):
  emb_p = emb[premise]; emb_h = emb[hypothesis]              # [B,T,E]
  premise LSTM (h0=c0=0) -> masked-blend cell -> c_last
  hypothesis LSTM (h0=0, c0=c_last) -> masked-blend hidden -> h_last
  logits = [h_last | similarity] @ fc_W.T + fc_b; out = log_softmax

Sharding: data-parallel over batch, 128 -> 16 per core on 8 cores.
Weights replicated. Everything on-device per core; host only
slices/concatenates per-core batch shards.

Device strategy per core:
  - token gather via indirect DMA (128 tokens per tile, 25 tiles)
  - f32->f16 cast + DMA-transpose to E-major for the input projection
  - input projection W_ih @ emb^T on PE (f16, fp32 accum), bias via ACT,
    result xg^T kept gate-major in SBUF (f16)
  - recurrence in gate-transposed layout: per step 64 weight-stationary
    [128,128] f16 matmuls (N=16) accumulate all 4 gates for the local
    batch into one PSUM bank [128, 256]; ACT sigmoid/tanh; DVE cell
    update + running masked-blend selection
  - tiny fp32 matmul + log_softmax head
"""

import os
import sys

import numpy as np

for _p in ("/opt/trn_rl_repo", "/root/.axon_site/_ro/trn_rl_repo"):
    if _p not in sys.path and os.path.isdir(_p):
        sys.path.insert(0, _p)

import concourse.bass as bass  # noqa: E402
from concourse import bacc  # noqa: E402
import concourse.mybir as mybir  # noqa: E402
import concourse.tile as tile  # noqa: E402
from concourse.bass import IndirectOffsetOnAxis, AP  # noqa: E402
from concourse.bass_utils import run_bass_kernel_spmd  # noqa: E402
from concourse.masks import make_identity  # noqa: E402

F32 = mybir.dt.float32
F16 = mybir.dt.float16
I32 = mybir.dt.int32

B, T, V, E, H, C = 128, 200, 50000, 300, 512, 3
NCORES = 8
BL = B // NCORES          # 16 local batch
G4 = 4 * H                # 2048 gates
NM = G4 // 128            # 16 gate chunks (m-tiles)
NKH = H // 128            # 4 h-dim k-tiles
EP = 384                  # E padded to 3*128
EK = EP // 128            # 3 E k-tiles
NTOK = BL * T             # 3200 tokens per core
NG = NTOK // 128          # 25 gather tiles
AF = mybir.ActivationFunctionType
ALU = mybir.AluOpType


def _load_proj_weights(nc, pools, wih_dram, wihT, ident):
    """Build W_ih^T f16 k-major tiles: wihT[:, k*G4 + m*128 + j] =
    W_ih[m*128 + j, k*128 + r] for partition r. Rows E..EP are zero."""
    for m in range(NM):
        w32 = pools["w32"].tile([128, EP], F32, tag="w32")
        nc.gpsimd.memset(w32[:, E:EP], 0.0)
        nc.sync.dma_start(out=w32[:, :E], in_=wih_dram[m * 128:(m + 1) * 128, :])
        for k in range(EK):
            tp = pools["tps"].tile([128, 128], F32, tag="tps")
            nc.tensor.transpose(
                out=tp[:], in_=w32[:, k * 128:(k + 1) * 128], identity=ident[:]
            )
            nc.vector.tensor_copy(
                out=wihT[:, k * G4 + m * 128:k * G4 + (m + 1) * 128], in_=tp[:]
            )


def _load_rec_weights(nc, pools, whh_dram, whhT, ident):
    """Build W_hh^T f16 tiles: whhT[:, k*G4 + m*128 + j] =
    W_hh[m*128 + j, k*128 + r] for partition r."""
    for m in range(NM):
        w32 = pools["w32"].tile([128, H], F32, tag="w32")
        nc.sync.dma_start(out=w32[:], in_=whh_dram[m * 128:(m + 1) * 128, :])
        for k in range(NKH):
            tp = pools["tps"].tile([128, 128], F32, tag="tps")
            nc.tensor.transpose(
                out=tp[:], in_=w32[:, k * 128:(k + 1) * 128], identity=ident[:]
            )
            nc.vector.tensor_copy(
                out=whhT[:, k * G4 + m * 128:k * G4 + (m + 1) * 128], in_=tp[:]
            )


def _load_bias(nc, pools, bih_dram, bhh_dram, ident):
    """bias[128, 16] f32: col m holds (bih+bhh)[m*128 : (m+1)*128]."""
    b0 = pools["w32"].tile([16, 128], F32, tag="b0")
    b1 = pools["w32"].tile([16, 128], F32, tag="b1")
    nc.sync.dma_start(out=b0[:], in_=bih_dram.rearrange("(m x) -> m x", m=NM))
    nc.sync.dma_start(out=b1[:], in_=bhh_dram.rearrange("(m x) -> m x", m=NM))
    bs = pools["w32"].tile([16, 128], F32, tag="bs")
    nc.vector.tensor_add(out=bs[:], in0=b0[:], in1=b1[:])
    bps = pools["mps"].tile([128, 16], F32, tag="mps")
    nc.tensor.transpose(out=bps[:], in_=bs[:], identity=ident[:16, :16])
    bias = pools["persist"].tile([128, 16], F32, tag=f"bias{bih_dram.tensor.name}")
    nc.vector.tensor_copy(out=bias[:], in_=bps[:])
    return bias


def _build_mask(nc, pools, tok_dram):
    """mask[128, 3200] f32 broadcast over partitions; col b*T+t = (tok[b,t]!=0).

    Built in place: tokens land in partition 0 (int32 view), are broadcast
    to all partitions, then compared against 0 in place.
    """
    mask = pools["mask"].tile([128, NTOK], I32, tag="mask")
    nc.sync.dma_start(out=mask[0:1, :], in_=tok_dram.rearrange("b t -> (b t)")[None, :])
    nc.gpsimd.partition_broadcast(mask[:], mask[0:1, :])
    nc.vector.tensor_scalar(
        out=mask[:], in0=mask[:], scalar1=0, scalar2=None, op0=ALU.not_equal
    )
    return mask


def _nsizes():
    sizes = []
    off = 0
    while off < NTOK:
        sizes.append(min(512, NTOK - off))
        off += 512
    return sizes


def _gather(nc, pools, tok_dram, emb_dram, ident):
    """Gather + transpose embeddings into E-major n-chunk tiles.

    Returns a list of (rhsT tile, nw). rhsT col k*512 + g*128 + q holds
    emb[token p*25+g][k*128 + row] for output col index q=p.
    """
    tokt = pools["tok"].tile([128, NG], I32, tag="tok")
    nc.sync.dma_start(
        out=tokt[:], in_=tok_dram.rearrange("b t -> (b t)").rearrange("(p g) -> p g", p=128)
    )
    chunks = []
    for n, nw in enumerate(_nsizes()):
        rhsT = pools["rhsT"].tile([128, EK * 512], F16, tag="rhsT")
        for gg in range(nw // 128):
            g = n * 4 + gg
            e32 = pools["emb32"].tile([128, EP], F32, tag="emb32")
            nc.gpsimd.memset(e32[:, E:EP], 0.0)
            nc.gpsimd.indirect_dma_start(
                out=e32[:, :E],
                out_offset=None,
                in_=emb_dram[:],
                in_offset=IndirectOffsetOnAxis(ap=tokt[:, g:g + 1], axis=0),
            )
            for k in range(EK):
                tp = pools["tps"].tile([128, 128], F32, tag="tps")
                nc.tensor.transpose(
                    out=tp[:], in_=e32[:, k * 128:(k + 1) * 128], identity=ident[:]
                )
                nc.vector.tensor_copy(
                    out=rhsT[:, k * 512 + gg * 128:k * 512 + (gg + 1) * 128],
                    in_=tp[:],
                )
        chunks.append((rhsT, nw))
    return chunks


def _proj_mms(nc, pools, chunks, wihT, bias, xg):
    """Input projection: xg[:, m*NTOK + col] = (W_ih @ emb^T + bias)."""
    for n, (rhsT, nw) in enumerate(chunks):
        for m in range(NM):
            pj = pools["pj"].tile([128, 512], F32, tag="pj")
            for k in range(EK):
                nc.tensor.matmul(
                    out=pj[:, :nw],
                    lhsT=wihT[:, k * G4 + m * 128:k * G4 + (m + 1) * 128],
                    rhs=rhsT[:, k * 512:k * 512 + nw],
                    start=(k == 0),
                    stop=(k == EK - 1),
                )
            nc.scalar.activation(
                out=xg[:, m * NTOK + n * 512:m * NTOK + n * 512 + nw],
                in_=pj[:, :nw],
                func=AF.Identity,
                bias=bias[:, m:m + 1],
            )


def _recurrence(nc, pools, whhT, xg, mask, c_init, blend_on, sel_tag, t_steps):
    """Run LSTM recurrence; returns running masked selection tile [128, 64].

    State layout: h^T/c^T as [128, 4*16]: partition r, col k*16+b holds
    state[k*128 + r, b]. Gates psum [128, 256]: col m*16+b.
    """
    sel_dt = F32 if blend_on == "c" else F16
    sel = pools["sel"].tile([128, NKH * BL], sel_dt, tag=sel_tag)
    nc.vector.memset(sel[:], 0.0)
    h16 = pools["h16"].tile([128, NKH * BL], F16, tag="h16")
    nc.vector.memset(h16[:], 0.0)
    if c_init is None:
        c = pools["cst"].tile([128, NKH * BL], F32, tag="cst")
        nc.vector.memset(c[:], 0.0)
    else:
        c = c_init

    for t in range(t_steps):
        # i/f/g gates in one PSUM bank, o in another: lets the i/f/g
        # elementwise chain start while PE still streams the o matmuls.
        gifg = pools["gifg"].tile([128, 12 * BL], F32, tag="gifg")
        gto = pools["gto"].tile([128, 4 * BL], F32, tag="gto")
        for m in range(NM):
            dst = (gifg[:, m * BL:(m + 1) * BL] if m < 12
                   else gto[:, (m - 12) * BL:(m - 11) * BL])
            for k in range(NKH):
                nc.tensor.matmul(
                    out=dst,
                    lhsT=whhT[:, k * G4 + m * 128:k * G4 + (m + 1) * 128],
                    rhs=h16[:, k * BL:(k + 1) * BL],
                    start=(k == 0),
                    stop=(k == NKH - 1),
                )
        # xg step slice: for gate chunk m, batch b: col m*NTOK + colbase + 8*b
        colbase = (t % NG) * 128 + t // NG
        cend = colbase + 8 * (BL - 1) + 1
        xg3 = xg[:].rearrange("p (m q) -> p m q", m=NM)
        gs = pools["gif"].tile([128, 12 * BL], F32, tag="gs")
        nc.vector.tensor_add(
            out=gs[:].rearrange("p (m b) -> p m b", m=12),
            in0=gifg[:].rearrange("p (m b) -> p m b", m=12),
            in1=xg3[:, 0:12, colbase:cend:8],
        )
        go = pools["t64"].tile([128, 64], F32, tag="go")
        nc.vector.tensor_add(
            out=go[:].rearrange("p (m b) -> p m b", m=4),
            in0=gto[:].rearrange("p (m b) -> p m b", m=4),
            in1=xg3[:, 12:16, colbase:cend:8],
        )
        sig_if = pools["sig"].tile([128, 8 * BL], F32, tag="sig_if")
        nc.scalar.activation(out=sig_if[:], in_=gs[:, 0:128], func=AF.Sigmoid)
        tng = pools["t64"].tile([128, 64], F32, tag="tng")
        nc.scalar.activation(out=tng[:], in_=gs[:, 128:192], func=AF.Tanh)
        tfc = pools["t64"].tile([128, 64], F32, tag="tfc")
        nc.vector.tensor_mul(out=tfc[:], in0=sig_if[:, 64:128], in1=c[:])
        tig = pools["t64"].tile([128, 64], F32, tag="tig")
        nc.vector.tensor_mul(out=tig[:], in0=sig_if[:, 0:64], in1=tng[:])
        c = pools["cst"].tile([128, NKH * BL], F32, tag="cst")
        nc.vector.tensor_add(out=c[:], in0=tfc[:], in1=tig[:])
        tnc = pools["t64"].tile([128, 64], F32, tag="tnc")
        nc.scalar.activation(out=tnc[:], in_=c[:], func=AF.Tanh)
        sgo = pools["t64"].tile([128, 64], F32, tag="sgo")
        nc.scalar.activation(out=sgo[:], in_=go[:], func=AF.Sigmoid)
        h16 = pools["h16"].tile([128, NKH * BL], F16, tag="h16")
        nc.vector.tensor_mul(out=h16[:], in0=sgo[:], in1=tnc[:])

        # running masked blend (mask is exactly 0/1): sel = m ? src : sel
        src = c if blend_on == "c" else h16
        mslice = mask[:, t:t + 1]
        mbc = AP(mslice.tensor, mslice.offset, [mslice.ap[0], [0, NKH], [T, BL]])
        nc.vector.copy_predicated(
            out=sel[:].rearrange("p (j b) -> p j b", j=NKH),
            mask=mbc,
            data=src[:].rearrange("p (j b) -> p j b", j=NKH),
        )
    return sel


def _head(nc, pools, sel_h, fcw_dram, fcb_dram, sim_dram, ident, out_dram):
    """logits[16,3] = [sel_h | sim | 1] @ [fc_W | fc_b]^T, then log_softmax."""
    fcw = pools["w32"].tile([C, H + 1], F32, tag="fcw")
    nc.sync.dma_start(out=fcw[:], in_=fcw_dram[:])
    fcwT = pools["persist"].tile([128, NKH * C], F16, tag="fcwT")
    for j in range(NKH):
        tp = pools["mps"].tile([128, C], F32, tag="mps")
        nc.tensor.transpose(
            out=tp[:], in_=fcw[:, j * 128:(j + 1) * 128], identity=ident[:C, :C]
        )
        nc.vector.tensor_copy(out=fcwT[:, j * C:(j + 1) * C], in_=tp[:])
    rhs45 = pools["persist"].tile([2, C], F32, tag="rhs45")
    nc.sync.dma_start(out=rhs45[0:1, :], in_=fcw_dram[:, H:H + 1].rearrange("a b -> b a"))
    nc.sync.dma_start(out=rhs45[1:2, :], in_=fcb_dram[None, :])
    lhsT45 = pools["persist"].tile([2, BL], F32, tag="lhsT45")
    nc.gpsimd.memset(lhsT45[:], 1.0)
    nc.sync.dma_start(out=lhsT45[0:1, :], in_=sim_dram.rearrange("a b -> b a"))

    lps = pools["mps"].tile([BL, C], F32, tag="mps")
    for j in range(NKH):
        nc.tensor.matmul(
            out=lps[:],
            lhsT=sel_h[:, j * BL:(j + 1) * BL],
            rhs=fcwT[:, j * C:(j + 1) * C],
            start=(j == 0),
            stop=False,
        )
    nc.tensor.matmul(out=lps[:], lhsT=lhsT45[:], rhs=rhs45[:], start=False, stop=True)

    mx = pools["head"].tile([BL, 1], F32, tag="mx")
    nc.vector.tensor_reduce(out=mx[:], in_=lps[:], axis=mybir.AxisListType.X, op=ALU.max)
    ls = pools["head"].tile([BL, C], F32, tag="ls")
    nc.vector.tensor_scalar(
        out=ls[:], in0=lps[:], scalar1=mx[:, 0:1], scalar2=None, op0=ALU.subtract
    )
    ex = pools["head"].tile([BL, C], F32, tag="ex")
    nc.scalar.activation(out=ex[:], in_=ls[:], func=AF.Exp)
    sm = pools["head"].tile([BL, 1], F32, tag="sm")
    nc.vector.tensor_reduce(out=sm[:], in_=ex[:], axis=mybir.AxisListType.X, op=ALU.add)
    lg = pools["head"].tile([BL, 1], F32, tag="lg")
    nc.scalar.activation(out=lg[:], in_=sm[:], func=AF.Ln)
    res = pools["head"].tile([BL, C], F32, tag="res")
    nc.vector.tensor_scalar(
        out=res[:], in0=ls[:], scalar1=lg[:, 0:1], scalar2=None, op0=ALU.subtract
    )
    nc.sync.dma_start(out=out_dram[:], in_=res[:])


def build(t_steps=T):
    nc = bacc.Bacc(
        "TRN2", target_bir_lowering=False, debug=False,
        enable_asserts=True, num_devices=NCORES,
    )
    prem = nc.declare_dram_parameter("premise", [BL, T], I32, isOutput=False)
    hyp = nc.declare_dram_parameter("hypothesis", [BL, T], I32, isOutput=False)
    sim = nc.declare_dram_parameter("similarity", [BL, 1], F32, isOutput=False)
    embw = nc.declare_dram_parameter("emb_weight", [V, E], F32, isOutput=False)
    wih_p = nc.declare_dram_parameter("Wih_p", [G4, E], F32, isOutput=False)
    whh_p = nc.declare_dram_parameter("Whh_p", [G4, H], F32, isOutput=False)
    bih_p = nc.declare_dram_parameter("bih_p", [G4], F32, isOutput=False)
    bhh_p = nc.declare_dram_parameter("bhh_p", [G4], F32, isOutput=False)
    wih_h = nc.declare_dram_parameter("Wih_h", [G4, E], F32, isOutput=False)
    whh_h = nc.declare_dram_parameter("Whh_h", [G4, H], F32, isOutput=False)
    bih_h = nc.declare_dram_parameter("bih_h", [G4], F32, isOutput=False)
    bhh_h = nc.declare_dram_parameter("bhh_h", [G4], F32, isOutput=False)
    fcw = nc.declare_dram_parameter("fc_W", [C, H + 1], F32, isOutput=False)
    fcb = nc.declare_dram_parameter("fc_b", [C], F32, isOutput=False)
    out = nc.declare_dram_parameter("out", [BL, C], F32, isOutput=True)

    with tile.TileContext(nc) as tc:
        from contextlib import ExitStack

        with ExitStack() as ctx:
            pools = {}

            def pool(name, bufs, space="SBUF"):
                pools[name] = ctx.enter_context(
                    tc.tile_pool(name=name, bufs=bufs, space=space)
                )

            pool("persist", 1)
            pool("w32", 2)
            pool("mask", 1)
            pool("tok", 2)
            pool("emb32", 3)
            pool("rhsT", 7)
            pool("xg", 1)
            pool("wihT", 1)
            pool("sel", 1)
            pool("h16", 2)
            pool("cst", 2)
            pool("gif", 2)
            pool("sig", 2)
            pool("t64", 3)
            pool("head", 1)
            pool("pj", 2, space="PSUM")
            pool("tps", 1, space="PSUM")
            pool("gifg", 2, space="PSUM")
            pool("gto", 2, space="PSUM")
            pool("mps", 1, space="PSUM")

            ident = pools["persist"].tile([128, 128], F32, tag="ident")
            make_identity(nc, ident[:])

            whhT_p = pools["persist"].tile([128, NKH * G4], F16, tag="whhT_p")
            whhT_h = pools["persist"].tile([128, NKH * G4], F16, tag="whhT_h")
            _load_rec_weights(nc, pools, whh_p, whhT_p, ident)
            _load_rec_weights(nc, pools, whh_h, whhT_h, ident)
            bias_p = _load_bias(nc, pools, bih_p[:], bhh_p[:], ident)
            bias_h = _load_bias(nc, pools, bih_h[:], bhh_h[:], ident)

            # ---- premise LSTM ----
            wihT = pools["wihT"].tile([128, EK * G4], F16, tag="wihT")
            _load_proj_weights(nc, pools, wih_p, wihT, ident)
            chunks = _gather(nc, pools, prem, embw, ident)
            xg = pools["xg"].tile([128, NM * NTOK], F16, tag="xg")
            _proj_mms(nc, pools, chunks, wihT, bias_p, xg)
            mask = _build_mask(nc, pools, prem)

            # hypothesis prefetch: W_ih^T load + gathers/transposes fill
            # the PE gaps of the premise recurrence
            wihT2 = pools["wihT"].tile([128, EK * G4], F16, tag="wihT")
            _load_proj_weights(nc, pools, wih_h, wihT2, ident)
            chunks2 = _gather(nc, pools, hyp, embw, ident)

            sel_c = _recurrence(
                nc, pools, whhT_p, xg, mask, None, "c", "sel_c", t_steps
            )

            # ---- hypothesis LSTM ----
            xg = pools["xg"].tile([128, NM * NTOK], F16, tag="xg")
            _proj_mms(nc, pools, chunks2, wihT2, bias_h, xg)
            mask = _build_mask(nc, pools, hyp)
            sel_h = _recurrence(
                nc, pools, whhT_h, xg, mask, sel_c, "h", "sel_h", t_steps
            )

            _head(nc, pools, sel_h, fcw, fcb, sim, ident, out)
    nc.compile()
    return nc


_NC_CACHE = {}


def _get_nc(t_steps=T):
    if t_steps not in _NC_CACHE:
        _NC_CACHE[t_steps] = build(t_steps)
    return _NC_CACHE[t_steps]


def kernel(**inputs):
    nc = _get_nc()
    prem = np.ascontiguousarray(np.asarray(inputs["premise"], dtype=np.int32))
    hyp = np.ascontiguousarray(np.asarray(inputs["hypothesis"], dtype=np.int32))
    sim = np.ascontiguousarray(np.asarray(inputs["similarity"], dtype=np.float32))
    shared = {
        name: np.ascontiguousarray(np.asarray(inputs[name], dtype=np.float32))
        for name in (
            "emb_weight", "Wih_p", "Whh_p", "bih_p", "bhh_p",
            "Wih_h", "Whh_h", "bih_h", "bhh_h", "fc_W", "fc_b",
        )
    }
    in_maps = []
    for i in range(NCORES):
        s = slice(i * BL, (i + 1) * BL)
        in_maps.append({"premise": prem[s], "hypothesis": hyp[s],
                        "similarity": sim[s], **shared})
    res = run_bass_kernel_spmd(nc, in_maps, list(range(NCORES)))
    return np.concatenate([res.results[i]["out"] for i in range(NCORES)], axis=0)


if __name__ == "__main__":
    rng = np.random.default_rng(0)
    ins = {
        "premise": rng.integers(0, V, (B, T)).astype(np.int32),
        "hypothesis": rng.integers(0, V, (B, T)).astype(np.int32),
        "similarity": rng.random((B, 1), dtype=np.float32),
        "emb_weight": rng.standard_normal((V, E), dtype=np.float32),
        "Wih_p": rng.standard_normal((G4, E), dtype=np.float32) * 0.04,
        "Whh_p": rng.standard_normal((G4, H), dtype=np.float32) * 0.04,
        "bih_p": rng.standard_normal(G4).astype(np.float32) * 0.04,
        "bhh_p": rng.standard_normal(G4).astype(np.float32) * 0.04,
        "Wih_h": rng.standard_normal((G4, E), dtype=np.float32) * 0.04,
        "Whh_h": rng.standard_normal((G4, H), dtype=np.float32) * 0.04,
        "bih_h": rng.standard_normal(G4).astype(np.float32) * 0.04,
        "bhh_h": rng.standard_normal(G4).astype(np.float32) * 0.04,
        "fc_W": rng.standard_normal((C, H + 1)).astype(np.float32) * 0.02,
        "fc_b": np.zeros(C, dtype=np.float32),
    }
    print(kernel(**ins).shape)


# revision 18
# speedup vs baseline: 1.1223x; 1.1223x over previous
"""Trainium2 Bass kernel for a two-LSTM premise/hypothesis classifier.

Model (per reference):
  emb_p = emb[premise]; emb_h = emb[hypothesis]              # [B,T,E]
  premise LSTM (h0=c0=0) -> masked-blend cell -> c_last
  hypothesis LSTM (h0=0, c0=c_last) -> masked-blend hidden -> h_last
  logits = [h_last | similarity] @ fc_W.T + fc_b; out = log_softmax

Sharding: data-parallel over batch, 128 -> 16 per core on 8 cores.
Weights replicated. Everything on-device per core; host only
slices/concatenates per-core batch shards.

Device strategy per core:
  - token gather via indirect DMA (128 tokens per tile, 25 tiles)
  - f32->f16 cast + DMA-transpose to E-major for the input projection
  - input projection W_ih @ emb^T on PE (f16, fp32 accum), bias via ACT,
    result xg^T kept gate-major in SBUF (f16)
  - recurrence in gate-transposed layout: per step 64 weight-stationary
    [128,128] f16 matmuls (N=16) accumulate all 4 gates for the local
    batch into one PSUM bank [128, 256]; ACT sigmoid/tanh; DVE cell
    update + running masked-blend selection
  - tiny fp32 matmul + log_softmax head
"""

import os
import sys

import numpy as np

for _p in ("/opt/trn_rl_repo", "/root/.axon_site/_ro/trn_rl_repo"):
    if _p not in sys.path and os.path.isdir(_p):
        sys.path.insert(0, _p)

import concourse.bass as bass  # noqa: E402
from concourse import bacc  # noqa: E402
import concourse.mybir as mybir  # noqa: E402
import concourse.tile as tile  # noqa: E402
from concourse.bass import IndirectOffsetOnAxis, AP  # noqa: E402
from concourse.bass_utils import run_bass_kernel_spmd  # noqa: E402
from concourse.masks import make_identity  # noqa: E402

F32 = mybir.dt.float32
F16 = mybir.dt.float16
I32 = mybir.dt.int32

B, T, V, E, H, C = 128, 200, 50000, 300, 512, 3
NCORES = 8
BL = B // NCORES          # 16 local batch
G4 = 4 * H                # 2048 gates
NM = G4 // 128            # 16 gate chunks (m-tiles)
NKH = H // 128            # 4 h-dim k-tiles
EP = 384                  # E padded to 3*128
EK = EP // 128            # 3 E k-tiles
NTOK = BL * T             # 3200 tokens per core
NG = NTOK // 128          # 25 gather tiles
AF = mybir.ActivationFunctionType
ALU = mybir.AluOpType


def _load_proj_weights(nc, pools, wih_dram, wihT, ident):
    """Build W_ih^T f16 k-major tiles: wihT[:, k*G4 + m*128 + j] =
    W_ih[m*128 + j, k*128 + r] for partition r. Rows E..EP are zero."""
    for m in range(NM):
        w32 = pools["w32"].tile([128, EP], F32, tag="w32")
        nc.gpsimd.memset(w32[:, E:EP], 0.0)
        nc.sync.dma_start(out=w32[:, :E], in_=wih_dram[m * 128:(m + 1) * 128, :])
        for k in range(EK):
            tp = pools["tps"].tile([128, 128], F32, tag="tps")
            nc.tensor.transpose(
                out=tp[:], in_=w32[:, k * 128:(k + 1) * 128], identity=ident[:]
            )
            nc.vector.tensor_copy(
                out=wihT[:, k * G4 + m * 128:k * G4 + (m + 1) * 128], in_=tp[:]
            )


def _load_rec_weights(nc, pools, whh_dram, whhT, ident):
    """Build W_hh^T f16 tiles: whhT[:, k*G4 + m*128 + j] =
    W_hh[m*128 + j, k*128 + r] for partition r."""
    for m in range(NM):
        w32 = pools["w32"].tile([128, H], F32, tag="w32")
        nc.sync.dma_start(out=w32[:], in_=whh_dram[m * 128:(m + 1) * 128, :])
        for k in range(NKH):
            tp = pools["tps"].tile([128, 128], F32, tag="tps")
            nc.tensor.transpose(
                out=tp[:], in_=w32[:, k * 128:(k + 1) * 128], identity=ident[:]
            )
            nc.vector.tensor_copy(
                out=whhT[:, k * G4 + m * 128:k * G4 + (m + 1) * 128], in_=tp[:]
            )


def _load_bias(nc, pools, bih_dram, bhh_dram, ident):
    """bias[128, 16] f32: col m holds (bih+bhh)[m*128 : (m+1)*128]."""
    b0 = pools["w32"].tile([16, 128], F32, tag="b0")
    b1 = pools["w32"].tile([16, 128], F32, tag="b1")
    nc.sync.dma_start(out=b0[:], in_=bih_dram.rearrange("(m x) -> m x", m=NM))
    nc.sync.dma_start(out=b1[:], in_=bhh_dram.rearrange("(m x) -> m x", m=NM))
    bs = pools["w32"].tile([16, 128], F32, tag="bs")
    nc.vector.tensor_add(out=bs[:], in0=b0[:], in1=b1[:])
    bps = pools["mps"].tile([128, 16], F32, tag="mps")
    nc.tensor.transpose(out=bps[:], in_=bs[:], identity=ident[:16, :16])
    bias = pools["persist"].tile([128, 16], F32, tag=f"bias{bih_dram.tensor.name}")
    nc.vector.tensor_copy(out=bias[:], in_=bps[:])
    return bias


def _build_mask(nc, pools, tok_dram):
    """mask[128, 3200] f32 broadcast over partitions; col b*T+t = (tok[b,t]!=0).

    Built in place: tokens land in partition 0 (int32 view), are broadcast
    to all partitions, then compared against 0 in place.
    """
    mask = pools["mask"].tile([128, NTOK], I32, tag="mask")
    nc.sync.dma_start(out=mask[0:1, :], in_=tok_dram.rearrange("b t -> (b t)")[None, :])
    nc.gpsimd.partition_broadcast(mask[:], mask[0:1, :])
    nc.vector.tensor_scalar(
        out=mask[:], in0=mask[:], scalar1=0, scalar2=None, op0=ALU.not_equal
    )
    return mask


def _nsizes():
    sizes = []
    off = 0
    while off < NTOK:
        sizes.append(min(512, NTOK - off))
        off += 512
    return sizes


def _gather(nc, pools, tok_dram, emb_dram, ident):
    """Gather + transpose embeddings into E-major n-chunk tiles.

    Returns a list of (rhsT tile, nw). rhsT col k*512 + g*128 + q holds
    emb[token p*25+g][k*128 + row] for output col index q=p.
    """
    tokt = pools["tok"].tile([128, NG], I32, tag="tok")
    nc.sync.dma_start(
        out=tokt[:], in_=tok_dram.rearrange("b t -> (b t)").rearrange("(p g) -> p g", p=128)
    )
    chunks = []
    for n, nw in enumerate(_nsizes()):
        rhsT = pools["rhsT"].tile([128, EK * 512], F16, tag="rhsT")
        for gg in range(nw // 128):
            g = n * 4 + gg
            e32 = pools["emb32"].tile([128, EP], F32, tag="emb32")
            nc.gpsimd.memset(e32[:, E:EP], 0.0)
            nc.gpsimd.indirect_dma_start(
                out=e32[:, :E],
                out_offset=None,
                in_=emb_dram[:],
                in_offset=IndirectOffsetOnAxis(ap=tokt[:, g:g + 1], axis=0),
            )
            for k in range(EK):
                tp = pools["tps"].tile([128, 128], F32, tag="tps")
                nc.tensor.transpose(
                    out=tp[:], in_=e32[:, k * 128:(k + 1) * 128], identity=ident[:]
                )
                nc.vector.tensor_copy(
                    out=rhsT[:, k * 512 + gg * 128:k * 512 + (gg + 1) * 128],
                    in_=tp[:],
                )
        chunks.append((rhsT, nw))
    return chunks


def _proj_mms(nc, pools, chunks, wihT, bias, xg):
    """Input projection: xg[:, m*NTOK + col] = (W_ih @ emb^T + bias)."""
    for n, (rhsT, nw) in enumerate(chunks):
        for m in range(NM):
            pj = pools["pj"].tile([128, 512], F32, tag="pj")
            for k in range(EK):
                nc.tensor.matmul(
                    out=pj[:, :nw],
                    lhsT=wihT[:, k * G4 + m * 128:k * G4 + (m + 1) * 128],
                    rhs=rhsT[:, k * 512:k * 512 + nw],
                    start=(k == 0),
                    stop=(k == EK - 1),
                )
            nc.scalar.activation(
                out=xg[:, m * NTOK + n * 512:m * NTOK + n * 512 + nw],
                in_=pj[:, :nw],
                func=AF.Identity,
                bias=bias[:, m:m + 1],
            )


def _recurrence(nc, pools, whhT, xg, mask, c_init, blend_on, sel_tag, t_steps):
    """Run LSTM recurrence; returns running masked selection tile [128, 64].

    State layout: h^T/c^T as [128, 4*16]: partition r, col k*16+b holds
    state[k*128 + r, b]. Gates psum [128, 256]: col m*16+b.
    """
    sel_dt = F32 if blend_on == "c" else F16
    sel = pools["sel"].tile([128, NKH * BL], sel_dt, tag=sel_tag)
    nc.vector.memset(sel[:], 0.0)
    h16 = pools["h16"].tile([128, NKH * BL], F16, tag="h16")
    nc.vector.memset(h16[:], 0.0)
    if c_init is None:
        c = pools["cst"].tile([128, NKH * BL], F32, tag="cst")
        nc.vector.memset(c[:], 0.0)
    else:
        c = c_init

    for t in range(t_steps):
        # one PSUM bank for all gates; Tile's range-precise deps let each
        # gate block's add start as soon as its own matmuls finish
        gt = pools["gt"].tile([128, NM * BL], F32, tag="gt")
        for m in range(NM):
            for k in range(NKH):
                nc.tensor.matmul(
                    out=gt[:, m * BL:(m + 1) * BL],
                    lhsT=whhT[:, k * G4 + m * 128:k * G4 + (m + 1) * 128],
                    rhs=h16[:, k * BL:(k + 1) * BL],
                    start=(k == 0),
                    stop=(k == NKH - 1),
                )
        # xg step slice: for gate chunk m, batch b: col m*NTOK + colbase + 8*b
        colbase = (t % NG) * 128 + t // NG
        cend = colbase + 8 * (BL - 1) + 1
        xg3 = xg[:].rearrange("p (m q) -> p m q", m=NM)
        gt3 = gt[:].rearrange("p (m b) -> p m b", m=NM)
        # engine FIFO order is chosen so each op's operands are the
        # earliest-ready: DVE add_if, add_g, tfc, add_o, tig, c, h16;
        # ACT sig_if, tng, sgo, tnc.
        if_s = pools["sig"].tile([128, 8 * BL], F32, tag="if_s")
        nc.vector.tensor_add(
            out=if_s[:].rearrange("p (m b) -> p m b", m=8),
            in0=gt3[:, 0:8, :],
            in1=xg3[:, 0:8, colbase:cend:8],
        )
        g_s = pools["t64"].tile([128, 64], F32, tag="g_s")
        nc.vector.tensor_add(
            out=g_s[:].rearrange("p (m b) -> p m b", m=4),
            in0=gt3[:, 8:12, :],
            in1=xg3[:, 8:12, colbase:cend:8],
        )
        sig_if = pools["sig"].tile([128, 8 * BL], F32, tag="sig_if")
        nc.scalar.activation(out=sig_if[:], in_=if_s[:], func=AF.Sigmoid)
        tng = pools["t64"].tile([128, 64], F32, tag="tng")
        nc.scalar.activation(out=tng[:], in_=g_s[:], func=AF.Tanh)
        tfc = pools["t64"].tile([128, 64], F32, tag="tfc")
        nc.vector.tensor_mul(out=tfc[:], in0=sig_if[:, 64:128], in1=c[:])
        o_s = pools["t64"].tile([128, 64], F32, tag="o_s")
        nc.vector.tensor_add(
            out=o_s[:].rearrange("p (m b) -> p m b", m=4),
            in0=gt3[:, 12:16, :],
            in1=xg3[:, 12:16, colbase:cend:8],
        )
        tig = pools["t64"].tile([128, 64], F32, tag="tig")
        nc.vector.tensor_mul(out=tig[:], in0=sig_if[:, 0:64], in1=tng[:])
        c = pools["cst"].tile([128, NKH * BL], F32, tag="cst")
        nc.vector.tensor_add(out=c[:], in0=tfc[:], in1=tig[:])
        sgo = pools["t64"].tile([128, 64], F32, tag="sgo")
        nc.scalar.activation(out=sgo[:], in_=o_s[:], func=AF.Sigmoid)
        tnc = pools["t64"].tile([128, 64], F32, tag="tnc")
        nc.scalar.activation(out=tnc[:], in_=c[:], func=AF.Tanh)
        h16 = pools["h16"].tile([128, NKH * BL], F16, tag="h16")
        nc.vector.tensor_mul(out=h16[:], in0=sgo[:], in1=tnc[:])

        # running masked blend (mask is exactly 0/1): sel = m ? src : sel
        src = c if blend_on == "c" else h16
        mslice = mask[:, t:t + 1]
        mbc = AP(mslice.tensor, mslice.offset, [mslice.ap[0], [0, NKH], [T, BL]])
        nc.vector.copy_predicated(
            out=sel[:].rearrange("p (j b) -> p j b", j=NKH),
            mask=mbc,
            data=src[:].rearrange("p (j b) -> p j b", j=NKH),
        )
    return sel


def _head(nc, pools, sel_h, fcw_dram, fcb_dram, sim_dram, ident, out_dram):
    """logits[16,3] = [sel_h | sim | 1] @ [fc_W | fc_b]^T, then log_softmax."""
    fcw = pools["w32"].tile([C, H + 1], F32, tag="fcw")
    nc.sync.dma_start(out=fcw[:], in_=fcw_dram[:])
    fcwT = pools["persist"].tile([128, NKH * C], F16, tag="fcwT")
    for j in range(NKH):
        tp = pools["mps"].tile([128, C], F32, tag="mps")
        nc.tensor.transpose(
            out=tp[:], in_=fcw[:, j * 128:(j + 1) * 128], identity=ident[:C, :C]
        )
        nc.vector.tensor_copy(out=fcwT[:, j * C:(j + 1) * C], in_=tp[:])
    rhs45 = pools["persist"].tile([2, C], F32, tag="rhs45")
    nc.sync.dma_start(out=rhs45[0:1, :], in_=fcw_dram[:, H:H + 1].rearrange("a b -> b a"))
    nc.sync.dma_start(out=rhs45[1:2, :], in_=fcb_dram[None, :])
    lhsT45 = pools["persist"].tile([2, BL], F32, tag="lhsT45")
    nc.gpsimd.memset(lhsT45[:], 1.0)
    nc.sync.dma_start(out=lhsT45[0:1, :], in_=sim_dram.rearrange("a b -> b a"))

    lps = pools["mps"].tile([BL, C], F32, tag="mps")
    for j in range(NKH):
        nc.tensor.matmul(
            out=lps[:],
            lhsT=sel_h[:, j * BL:(j + 1) * BL],
            rhs=fcwT[:, j * C:(j + 1) * C],
            start=(j == 0),
            stop=False,
        )
    nc.tensor.matmul(out=lps[:], lhsT=lhsT45[:], rhs=rhs45[:], start=False, stop=True)

    mx = pools["head"].tile([BL, 1], F32, tag="mx")
    nc.vector.tensor_reduce(out=mx[:], in_=lps[:], axis=mybir.AxisListType.X, op=ALU.max)
    ls = pools["head"].tile([BL, C], F32, tag="ls")
    nc.vector.tensor_scalar(
        out=ls[:], in0=lps[:], scalar1=mx[:, 0:1], scalar2=None, op0=ALU.subtract
    )
    ex = pools["head"].tile([BL, C], F32, tag="ex")
    nc.scalar.activation(out=ex[:], in_=ls[:], func=AF.Exp)
    sm = pools["head"].tile([BL, 1], F32, tag="sm")
    nc.vector.tensor_reduce(out=sm[:], in_=ex[:], axis=mybir.AxisListType.X, op=ALU.add)
    lg = pools["head"].tile([BL, 1], F32, tag="lg")
    nc.scalar.activation(out=lg[:], in_=sm[:], func=AF.Ln)
    res = pools["head"].tile([BL, C], F32, tag="res")
    nc.vector.tensor_scalar(
        out=res[:], in0=ls[:], scalar1=lg[:, 0:1], scalar2=None, op0=ALU.subtract
    )
    nc.sync.dma_start(out=out_dram[:], in_=res[:])


def build(t_steps=T):
    nc = bacc.Bacc(
        "TRN2", target_bir_lowering=False, debug=False,
        enable_asserts=True, num_devices=NCORES,
    )
    prem = nc.declare_dram_parameter("premise", [BL, T], I32, isOutput=False)
    hyp = nc.declare_dram_parameter("hypothesis", [BL, T], I32, isOutput=False)
    sim = nc.declare_dram_parameter("similarity", [BL, 1], F32, isOutput=False)
    embw = nc.declare_dram_parameter("emb_weight", [V, E], F32, isOutput=False)
    wih_p = nc.declare_dram_parameter("Wih_p", [G4, E], F32, isOutput=False)
    whh_p = nc.declare_dram_parameter("Whh_p", [G4, H], F32, isOutput=False)
    bih_p = nc.declare_dram_parameter("bih_p", [G4], F32, isOutput=False)
    bhh_p = nc.declare_dram_parameter("bhh_p", [G4], F32, isOutput=False)
    wih_h = nc.declare_dram_parameter("Wih_h", [G4, E], F32, isOutput=False)
    whh_h = nc.declare_dram_parameter("Whh_h", [G4, H], F32, isOutput=False)
    bih_h = nc.declare_dram_parameter("bih_h", [G4], F32, isOutput=False)
    bhh_h = nc.declare_dram_parameter("bhh_h", [G4], F32, isOutput=False)
    fcw = nc.declare_dram_parameter("fc_W", [C, H + 1], F32, isOutput=False)
    fcb = nc.declare_dram_parameter("fc_b", [C], F32, isOutput=False)
    out = nc.declare_dram_parameter("out", [BL, C], F32, isOutput=True)

    with tile.TileContext(nc) as tc:
        from contextlib import ExitStack

        with ExitStack() as ctx:
            pools = {}

            def pool(name, bufs, space="SBUF"):
                pools[name] = ctx.enter_context(
                    tc.tile_pool(name=name, bufs=bufs, space=space)
                )

            pool("persist", 1)
            pool("w32", 2)
            pool("mask", 1)
            pool("tok", 2)
            pool("emb32", 3)
            pool("rhsT", 7)
            pool("xg", 1)
            pool("wihT", 1)
            pool("sel", 1)
            pool("h16", 2)
            pool("cst", 2)
            pool("gif", 2)
            pool("sig", 2)
            pool("t64", 3)
            pool("head", 1)
            pool("pj", 2, space="PSUM")
            pool("tps", 2, space="PSUM")
            pool("gt", 2, space="PSUM")
            pool("mps", 1, space="PSUM")

            ident = pools["persist"].tile([128, 128], F32, tag="ident")
            make_identity(nc, ident[:])

            whhT_p = pools["persist"].tile([128, NKH * G4], F16, tag="whhT_p")
            whhT_h = pools["persist"].tile([128, NKH * G4], F16, tag="whhT_h")
            _load_rec_weights(nc, pools, whh_p, whhT_p, ident)
            _load_rec_weights(nc, pools, whh_h, whhT_h, ident)
            bias_p = _load_bias(nc, pools, bih_p[:], bhh_p[:], ident)
            bias_h = _load_bias(nc, pools, bih_h[:], bhh_h[:], ident)

            # ---- premise LSTM ----
            wihT = pools["wihT"].tile([128, EK * G4], F16, tag="wihT")
            _load_proj_weights(nc, pools, wih_p, wihT, ident)
            chunks = _gather(nc, pools, prem, embw, ident)
            xg = pools["xg"].tile([128, NM * NTOK], F16, tag="xg")
            _proj_mms(nc, pools, chunks, wihT, bias_p, xg)
            mask = _build_mask(nc, pools, prem)

            # hypothesis prefetch: W_ih^T load + gathers/transposes fill
            # the PE gaps of the premise recurrence
            wihT2 = pools["wihT"].tile([128, EK * G4], F16, tag="wihT")
            _load_proj_weights(nc, pools, wih_h, wihT2, ident)
            chunks2 = _gather(nc, pools, hyp, embw, ident)

            sel_c = _recurrence(
                nc, pools, whhT_p, xg, mask, None, "c", "sel_c", t_steps
            )

            # ---- hypothesis LSTM ----
            xg = pools["xg"].tile([128, NM * NTOK], F16, tag="xg")
            _proj_mms(nc, pools, chunks2, wihT2, bias_h, xg)
            mask = _build_mask(nc, pools, hyp)
            sel_h = _recurrence(
                nc, pools, whhT_h, xg, mask, sel_c, "h", "sel_h", t_steps
            )

            _head(nc, pools, sel_h, fcw, fcb, sim, ident, out)
    nc.compile()
    return nc


_NC_CACHE = {}


def _get_nc(t_steps=T):
    if t_steps not in _NC_CACHE:
        _NC_CACHE[t_steps] = build(t_steps)
    return _NC_CACHE[t_steps]


def kernel(**inputs):
    nc = _get_nc()
    prem = np.ascontiguousarray(np.asarray(inputs["premise"], dtype=np.int32))
    hyp = np.ascontiguousarray(np.asarray(inputs["hypothesis"], dtype=np.int32))
    sim = np.ascontiguousarray(np.asarray(inputs["similarity"], dtype=np.float32))
    shared = {
        name: np.ascontiguousarray(np.asarray(inputs[name], dtype=np.float32))
        for name in (
            "emb_weight", "Wih_p", "Whh_p", "bih_p", "bhh_p",
            "Wih_h", "Whh_h", "bih_h", "bhh_h", "fc_W", "fc_b",
        )
    }
    in_maps = []
    for i in range(NCORES):
        s = slice(i * BL, (i + 1) * BL)
        in_maps.append({"premise": prem[s], "hypothesis": hyp[s],
                        "similarity": sim[s], **shared})
    res = run_bass_kernel_spmd(nc, in_maps, list(range(NCORES)))
    return np.concatenate([res.results[i]["out"] for i in range(NCORES)], axis=0)


if __name__ == "__main__":
    rng = np.random.default_rng(0)
    ins = {
        "premise": rng.integers(0, V, (B, T)).astype(np.int32),
        "hypothesis": rng.integers(0, V, (B, T)).astype(np.int32),
        "similarity": rng.random((B, 1), dtype=np.float32),
        "emb_weight": rng.standard_normal((V, E), dtype=np.float32),
        "Wih_p": rng.standard_normal((G4, E), dtype=np.float32) * 0.04,
        "Whh_p": rng.standard_normal((G4, H), dtype=np.float32) * 0.04,
        "bih_p": rng.standard_normal(G4).astype(np.float32) * 0.04,
        "bhh_p": rng.standard_normal(G4).astype(np.float32) * 0.04,
        "Wih_h": rng.standard_normal((G4, E), dtype=np.float32) * 0.04,
        "Whh_h": rng.standard_normal((G4, H), dtype=np.float32) * 0.04,
        "bih_h": rng.standard_normal(G4).astype(np.float32) * 0.04,
        "bhh_h": rng.standard_normal(G4).astype(np.float32) * 0.04,
        "fc_W": rng.standard_normal((C, H + 1)).astype(np.float32) * 0.02,
        "fc_b": np.zeros(C, dtype=np.float32),
    }
    print(kernel(**ins).shape)
